# revision 17
# baseline (speedup 1.0000x reference)
"""Trainium2 Bass kernel for causal multi-head attention with RoPE.

Problem: B=2, S=2048, D=1024, H=16, HD=64, fp32, causal mask.
Sharding: 8 cores = 2 (batch) x 4 (head-groups of 4 heads). Each core
computes QKV projection for its 4 heads, RoPE, causal attention, and a
partial output projection; the host sums the 4 per-batch partials and
transposes.

Kernel design (per core, all matmuls fp32r = full PE rate, ~1e-4 rel):
- x^T resident streaming: QK projection produces pair-layout transposed
  Q/K tiles [128, 2048] (rows = 2 heads x 64 interleaved rope-dims); V
  projected in natural [s, d] layout into [V|1]-augmented tiles (M=65
  matmuls co-compute softmax denominators for free).
- RoPE via 4 DVE ops per tile-chunk using a parity-swap stream_shuffle
  and sign-baked sin tables (W columns pre-permuted on host so rope
  pairs are adjacent partitions).
- Attention: scores matmul pairs row-packed via tile_position (0,0) /
  (64,0); exp on ACT (scale=1/8 fused); causal diagonal handled with
  N-windowed matmuls + post-exp triangle zeroing via gpsimd
  affine_select; attn@V accumulates [65, 512] psum per head.
- Softmax: denominator row -> DVE reciprocal -> [33,512] selector
  matmul broadcast -> normalize fused into out-projection staging.
"""
import numpy as np

B, S, D, H = 2, 2048, 1024, 16
HD = D // H  # 64
NCORES = 8
HEADS_PER_CORE = 4
ROPE_BASE = 10000.0

_CACHE = {}


# ---------------------------------------------------------------------------
# TileContext workarounds for this container's walrus (1 sync-wait/inst cap)
# ---------------------------------------------------------------------------
def _make_tc_class():
    import bass_rust
    import concourse.mybir as mybir
    import concourse.tile as tile
    from concourse.vector_clock import ScopedClock, VectorClock

    def legalize_waits(nc):
        uid = 0
        for fn in nc.m.functions:
            for blk in fn.blocks:
                new_list = []
                for inst in blk.instructions:
                    si = inst.sync_info
                    waits = list(si.on_wait) if si and si.on_wait else []
                    cap = 2 if isinstance(inst, mybir.InstEventSemaphore) else 1
                    if len(waits) > cap:
                        keep, excess = waits[:cap], waits[cap:]
                        for w in excess:
                            uid += 1
                            nop = mybir.InstNoOp(
                                name=f"waitnop-{uid}-{inst.name}",
                                opcode="NoOp",
                                engine=inst.engine,
                                ins=[],
                                outs=[],
                                sync_info=bass_rust.SyncInfo(
                                    on_wait=[w], on_update=[]
                                ),
                                text_hint="split_wait",
                            )
                            new_list.append(nop)
                        si.on_wait = keep
                    new_list.append(inst)
                blk.instructions[:] = new_list

    class SplitDrainTileContext(tile.TileContext):
        def _drain_and_barrier(self, tick_clock, wait_clock):
            gc = tick_clock.global_clock
            nprocs = len(gc)
            for i in range(nprocs):
                t = gc[i]
                if t > 0:
                    nop_inst = self.nc.sync.nop(hint=f"tail_wait_p{i}", nofuse=True)
                    vec = [0] * nprocs
                    vec[i] = t
                    wait_clock.add_sem_waits(
                        nop_inst.ins, ScopedClock({None: VectorClock(vec)})
                    )
            self.nc.sync.drain()
            self.nc.all_engine_barrier()
            assert self.sems is not None
            popped = self.nc._tile_sem_poison_stack.pop()
            assert popped is self._sem_poison
            self.nc.clear_and_free_semaphores(list(self.sems.allocated().values()))
            self.nc.all_engine_barrier()

        def __exit__(self, *exc):
            ret = super().__exit__(*exc)
            if exc[0] is None:
                legalize_waits(self.nc)
            return ret

    return SplitDrainTileContext


# ---------------------------------------------------------------------------
# Bass kernel builder
# ---------------------------------------------------------------------------
def _build_nc(causal: bool, reps: int = 1, unroll: int = 1):
    import concourse.bass as bass
    import concourse.mybir as mybir

    dt = mybir.dt
    F32, F32R, BF16 = dt.float32, dt.float32r, dt.bfloat16
    AF = mybir.ActivationFunctionType
    TC = _make_tc_class()

    nc = bass.Bass(trn_type="TRN2", target_bir_lowering=False, debug=False)

    xT = nc.dram_tensor("xT", [D, S], F32R, kind="ExternalInput")
    wqk = nc.dram_tensor("wqk", [D, 512], F32R, kind="ExternalInput")
    wv = nc.dram_tensor("wv", [D, 256], F32R, kind="ExternalInput")
    wout = nc.dram_tensor("wout", [256, D], F32R, kind="ExternalInput")
    ctab = nc.dram_tensor("ctab", [128, S], F32, kind="ExternalInput")
    stab2 = nc.dram_tensor("stab2", [128, S], F32, kind="ExternalInput")
    seld = nc.dram_tensor("seld", [33, 128], F32R, kind="ExternalInput")
    onescol = nc.dram_tensor("onescol", [128, 4, 1], BF16, kind="ExternalInput")
    yT = nc.dram_tensor("yT", [D, S], F32, kind="ExternalOutput")

    NQC = S // 512  # 4 q-chunks
    NKT = S // 128  # 16 k-tiles
    SHUF_SWAP = [(i ^ 1) for i in range(32)]

    with TC(nc) as tc:
        from contextlib import ExitStack

        with ExitStack() as ctx:
            cst = ctx.enter_context(tc.tile_pool(name="cst", bufs=1))

            # --- persistent tiles
            wqk_sb = cst.tile([128, 8 * 512], F32R)
            nc.sync.dma_start(
                wqk_sb[:].rearrange("p (kt c) -> p kt c", kt=8),
                wqk.ap().rearrange("(kt p) c -> p kt c", p=128),
            )
            wv_sb = cst.tile([128, 8 * 256], F32R)
            nc.sync.dma_start(
                wv_sb[:].rearrange("p (kt c) -> p kt c", kt=8),
                wv.ap().rearrange("(kt p) c -> p kt c", p=128),
            )
            wout_sb = cst.tile([128, 2 * 1024], F32R)
            nc.sync.dma_start(
                wout_sb[:].rearrange("p (kt c) -> p kt c", kt=2),
                wout.ap().rearrange("(kt p) c -> p kt c", p=128),
            )
            ctab_sb = cst.tile([128, S], F32)
            nc.sync.dma_start(ctab_sb[:], ctab.ap())
            stab_sb = cst.tile([128, S], F32)
            nc.sync.dma_start(stab_sb[:], stab2.ap())
            sel_sb = cst.tile([33, 128], F32R)
            nc.sync.dma_start(sel_sb[:], seld.ap())
            recs = []
            for i in range(2):
                r = cst.tile([33, 512], F32R, name=f"rec{i}")
                nc.vector.memset(r[:].bitcast(F32), 0.0)
                recs.append(r)

            vaug = [
                cst.tile([128, 4 * 65], BF16, name=f"vaug{st}") for st in range(NKT)
            ]
            for st in range(NKT):
                nc.sync.dma_start(
                    vaug[st][:].rearrange("p (h c) -> p h c", h=4)[:, :, 64:65],
                    onescol.ap(),
                )

            # Q/K pair tiles (bf16): [q_p0, q_p1, k_p0, k_p1]
            qk_pair = [
                cst.tile([128, S], BF16, name=f"qk{i}") for i in range(4)
            ]

            assert reps % unroll == 0
            n_iter = reps // unroll
            loop_ctx = (
                tc.For_i(0, n_iter, 1, staggered_reset=True)
                if n_iter > 1 else None
            )
            if loop_ctx is not None:
                ctx.enter_context(loop_ctx)
            xt_pool = ctx.enter_context(tc.tile_pool(name="xt", bufs=12))
            rope_pool = ctx.enter_context(tc.tile_pool(name="rope", bufs=6))
            pT_pool = ctx.enter_context(tc.tile_pool(name="pT", bufs=8))
            stg_pool = ctx.enter_context(tc.tile_pool(name="stg", bufs=4))
            yev_pool = ctx.enter_context(tc.tile_pool(name="yev", bufs=4))
            # PSUM split (8 banks total) chosen so A(sc+1) QKV tiles have
            # slots free while B(sc) runs (A/B overlap):
            #   pu: u (QKV) + vps   [128,512]f32  = 1 bank  x2 = 2
            #   mm: scores/bcps/yps [128,1024]f32 = 2 banks x2 = 4
            #   oo: oAB accumulator [128,1024]f32 = 2 banks x1 = 2
            pu = ctx.enter_context(tc.tile_pool(name="pu", bufs=2, space="PSUM"))
            mm = ctx.enter_context(tc.tile_pool(name="mm", bufs=2, space="PSUM"))
            oo = ctx.enter_context(tc.tile_pool(name="oo", bufs=1, space="PSUM"))

            stgs = {}

            def phase_a(sc, pfx=""):
                chunks = []
                for kt in range(8):
                    xt = xt_pool.tile([128, 512], F32R, tag="xt", name=f"{pfx}xt{sc}_{kt}")
                    nc.sync.dma_start(
                        xt[:],
                        xT.ap()[kt * 128:(kt + 1) * 128, sc * 512:(sc + 1) * 512],
                    )
                    chunks.append(xt)
                ts = slice(sc * 512, (sc + 1) * 512)
                for T in range(4):
                    u = pu.tile([128, 512], F32, tag="pu", name=f"{pfx}u{sc}_{T}")
                    for kt in range(8):
                        off = kt * 512 + T * 128
                        nc.tensor.matmul(
                            u[:], wqk_sb[:, off:off + 128], chunks[kt][:],
                            start=(kt == 0), stop=(kt == 7),
                        )
                    m1 = rope_pool.tile([128, 512], F32, tag="m1", name=f"{pfx}m1_{sc}_{T}")
                    nc.vector.tensor_mul(m1[:], u[:], ctab_sb[:, ts])
                    m2p = rope_pool.tile([128, 512], F32, tag="m2p", name=f"{pfx}m2p{sc}_{T}")
                    nc.vector.tensor_mul(m2p[:], u[:], stab_sb[:, ts])
                    m2 = rope_pool.tile([128, 512], F32, tag="m2", name=f"{pfx}m2_{sc}_{T}")
                    nc.vector.stream_shuffle(m2[:], m2p[:], SHUF_SWAP)
                    nc.gpsimd.tensor_add(qk_pair[T][:, ts], m1[:], m2[:])
                for j in range(4):
                    st = 4 * sc + j
                    vps = pu.tile([128, 512], F32, tag="pu", name=f"{pfx}v{st}")
                    for kt in range(8):
                        nc.tensor.matmul(
                            vps[:, 0:256],
                            chunks[kt][:, j * 128:(j + 1) * 128],
                            wv_sb[:, kt * 256:(kt + 1) * 256],
                            start=(kt == 0), stop=(kt == 7),
                        )
                    nc.vector.tensor_copy(
                        vaug[st][:].rearrange("p (h c) -> p h c", h=4)[:, :, 0:64],
                        vps[:, 0:256].rearrange("p (h c) -> p h c", h=4),
                    )

            def phase_b(qc, pfx=""):
                qs = slice(qc * 512, (qc + 1) * 512)
                for p in range(2):
                    q_t, k_t = qk_pair[p], qk_pair[2 + p]
                    cA = (2 * p) * 65
                    cB = (2 * p + 1) * 65
                    oAB = oo.tile([128, 1024], F32, tag="oo", name=f"{pfx}o{qc}_{p}")
                    main_kts = list(range(4 * qc)) if causal else list(range(NKT))
                    n_av = len(main_kts) + (4 if causal else 0)
                    avi = 0
                    for kt in main_kts:
                        ks = slice(kt * 128, (kt + 1) * 128)
                        sAB = mm.tile([128, 1024], F32, tag="mm", name=f"{pfx}s{qc}_{p}_{kt}")
                        nc.tensor.matmul(
                            sAB[:, 0:512], k_t[0:64, ks], q_t[0:64, qs],
                            start=True, stop=True,
                        )
                        nc.tensor.matmul(
                            sAB[:, 512:1024], k_t[64:128, ks], q_t[64:128, qs],
                            start=True, stop=True, tile_position=(64, 0),
                        )
                        pT = pT_pool.tile([128, 1024], BF16, tag="pT", name=f"{pfx}p{qc}_{p}_{kt}")
                        nc.scalar.activation(pT[:], sAB[:], AF.Exp, scale=0.125)
                        last = avi == n_av - 1
                        nc.tensor.matmul(
                            oAB[0:65, 0:512], vaug[kt][:, cA:cA + 65],
                            pT[:, 0:512], start=(avi == 0), stop=last,
                        )
                        nc.tensor.matmul(
                            oAB[0:65, 512:1024], vaug[kt][:, cB:cB + 65],
                            pT[:, 512:1024], start=(avi == 0), stop=last,
                        )
                        avi += 1
                    if causal:
                        for dl in range(4):
                            kt = 4 * qc + dl
                            w = 512 - 128 * dl
                            ks = slice(kt * 128, (kt + 1) * 128)
                            qws = slice(qc * 512 + 128 * dl, (qc + 1) * 512)
                            dAB = mm.tile([128, 1024], F32, tag="mm", name=f"{pfx}d{qc}_{p}_{dl}")
                            nc.tensor.matmul(
                                dAB[:, 0:w], k_t[0:64, ks], q_t[0:64, qws],
                                start=True, stop=True,
                            )
                            nc.tensor.matmul(
                                dAB[:, 512:512 + w], k_t[64:128, ks],
                                q_t[64:128, qws],
                                start=True, stop=True, tile_position=(64, 0),
                            )
                            pT = pT_pool.tile([128, 1024], BF16, tag="pT", name=f"{pfx}pd{qc}_{p}_{dl}")
                            src = dAB[:].rearrange("p (b c) -> p b c", b=2)[:, :, 0:w]
                            dst = pT[:].rearrange("p (b c) -> p b c", b=2)[:, :, 128 * dl:512]
                            nc.scalar.activation(dst, src, AF.Exp, scale=0.125)
                            for base in (128 * dl, 512 + 128 * dl):
                                nc.gpsimd.affine_select(
                                    out=pT[:, base:base + 128],
                                    in_=pT[:, base:base + 128],
                                    compare_op=mybir.AluOpType.is_ge,
                                    fill=0.0, base=0,
                                    pattern=[[1, 128]], channel_multiplier=-1,
                                )
                            last = avi == n_av - 1
                            nc.tensor.matmul(
                                oAB[0:65, 128 * dl:512],
                                vaug[kt][:, cA:cA + 65],
                                pT[:, 128 * dl:512],
                                start=(avi == 0), stop=last,
                            )
                            nc.tensor.matmul(
                                oAB[0:65, 512 + 128 * dl:1024],
                                vaug[kt][:, cB:cB + 65],
                                pT[:, 512 + 128 * dl:1024],
                                start=(avi == 0), stop=last,
                            )
                            avi += 1
                    # softmax denominators -> reciprocal -> broadcast
                    rec = recs[p]
                    with nc.allow_low_precision(reason="softmax denom"):
                        nc.vector.reciprocal(rec[0:1, :], oAB[64:65, 0:512])
                        nc.vector.reciprocal(rec[32:33, :], oAB[64:65, 512:1024])
                    bcps = mm.tile([128, 1024], F32, tag="mm", name=f"{pfx}bc{qc}_{p}")
                    nc.tensor.matmul(
                        bcps[:, 0:512], sel_sb[:], rec[:], start=True, stop=True
                    )
                    bco = rope_pool.tile([128, 512], F32, tag="bco", name=f"{pfx}bco{qc}_{p}")
                    nc.vector.tensor_copy(bco[:], bcps[:, 0:512])
                    stg = stg_pool.tile([128, 512], F32R, tag="stg", name=f"{pfx}stg{qc}_{p}")
                    nc.vector.tensor_mul(stg[0:64, :], oAB[0:64, 0:512], bco[0:64, :])
                    nc.vector.tensor_mul(
                        stg[64:128, :], oAB[0:64, 512:1024], bco[64:128, :]
                    )
                    stgs[(qc, p)] = stg
                # ---- output projection for this q-chunk
                for dm in range(8):
                    yps = mm.tile([128, 1024], F32, tag="mm", name=f"{pfx}y{qc}_{dm}")
                    nc.tensor.matmul(
                        yps[:, 0:512], wout_sb[:, dm * 128:dm * 128 + 128],
                        stgs[(qc, 0)][:], start=True, stop=False,
                    )
                    nc.tensor.matmul(
                        yps[:, 0:512],
                        wout_sb[:, 1024 + dm * 128:1024 + dm * 128 + 128],
                        stgs[(qc, 1)][:], start=False, stop=True,
                    )
                    yev = yev_pool.tile([128, 512], F32, tag="yev", name=f"{pfx}ye{qc}_{dm}")
                    if dm % 2 == 0:
                        nc.scalar.copy(yev[:], yps[:, 0:512])
                    else:
                        nc.vector.tensor_copy(yev[:], yps[:, 0:512])
                    nc.sync.dma_start(
                        yT.ap()[dm * 128:(dm + 1) * 128, qc * 512:(qc + 1) * 512],
                        yev[:],
                    )

            for cp in range(unroll):
                pfx = f"c{cp}_" if unroll > 1 else ""
                if causal:
                    # Fused: B(sc) only needs K/V chunks <= sc, so A(sc+1)
                    # can overlap B(sc) (DVE-heavy rope vs ACT-heavy exp).
                    for sc in range(NQC):
                        phase_a(sc, pfx)
                        phase_b(sc, pfx)
                else:
                    for sc in range(NQC):
                        phase_a(sc, pfx)
                    for qc in range(NQC):
                        phase_b(qc, pfx)
    return nc


# ---------------------------------------------------------------------------
# Host-side prep / gather
# ---------------------------------------------------------------------------
def _rope_tables():
    inv_freq = 1.0 / (ROPE_BASE ** (np.arange(0, HD, 2, dtype=np.float64) / HD))
    pos = np.arange(S, dtype=np.float64)
    freqs = np.outer(inv_freq, pos)  # [32, S]
    cos, sin = np.cos(freqs), np.sin(freqs)
    # pair-tile rows: r = head-local interleaved dim; m = (r % 64) // 2
    ctab = np.empty((128, S), np.float32)
    stab2 = np.empty((128, S), np.float32)
    for r in range(128):
        m = (r % 64) // 2
        ctab[r] = cos[m]
        # S[r] = -sin if r even else +sin ; stab2[r] = S[r^1]
        stab2[r] = sin[m] if (r % 2 == 0) else -sin[m]
    return ctab, stab2


def _prep_core_inputs(x, Wqkv, Wout):
    """Returns list of 8 in_map dicts."""
    perm = np.empty(HD, np.int64)
    perm[0::2] = np.arange(32)
    perm[1::2] = np.arange(32, 64)
    ctab, stab2 = _rope_tables()
    import ml_dtypes

    sel = np.zeros((33, 128), np.float32)
    sel[0, 0:64] = 1.0
    sel[32, 64:128] = 1.0
    onescol = np.ones((128, 4, 1), ml_dtypes.bfloat16)

    xT_b = [np.ascontiguousarray(x[b].T) for b in range(B)]

    in_maps = []
    for core in range(NCORES):
        b, g = divmod(core, 4)
        heads = [4 * g + j for j in range(HEADS_PER_CORE)]
        qcols = np.concatenate([h * HD + perm for h in heads])
        kcols = D + qcols
        vcols = 2 * D + np.concatenate(
            [h * HD + np.arange(HD) for h in heads]
        )
        wqk = np.ascontiguousarray(
            np.concatenate(
                [Wqkv[:, qcols], Wqkv[:, kcols]], axis=1
            )
        )  # [D, 512]
        wv = np.ascontiguousarray(Wqkv[:, vcols])  # [D, 256]
        orows = np.concatenate([h * HD + np.arange(HD) for h in heads])
        wout_c = np.ascontiguousarray(Wout[orows, :])  # [256, D]
        in_maps.append({
            "xT": xT_b[b],
            "wqk": wqk,
            "wv": wv,
            "wout": wout_c,
            "ctab": ctab,
            "stab2": stab2,
            "seld": sel,
            "onescol": onescol,
        })
    return in_maps


def _gather(results):
    y = np.empty((B, S, D), np.float32)
    for b in range(B):
        acc = results[4 * b]["yT"].astype(np.float64)
        for g in range(1, 4):
            acc += results[4 * b + g]["yT"]
        y[b] = acc.T.astype(np.float32)
    return y


def _mask_kind(mask):
    m = np.asarray(mask).reshape(S, S)
    if m.all():
        return "full"
    tri = np.tril(np.ones((S, S), dtype=bool))
    if (m == tri).all():
        return "causal"
    raise NotImplementedError("only causal (tril) or all-ones masks supported")


def _get_nc(causal, reps=1, unroll=1):
    key = ("nc", causal, reps, unroll)
    if key not in _CACHE:
        _CACHE[key] = _build_nc(causal, reps, unroll)
    return _CACHE[key]


def kernel(x, Wqkv, Wout, mask):
    from concourse.bass_utils import run_bass_kernel_spmd

    x = np.asarray(x, dtype=np.float32)
    Wqkv = np.asarray(Wqkv, dtype=np.float32)
    Wout = np.asarray(Wout, dtype=np.float32)
    causal = _mask_kind(mask) == "causal"

    nc = _get_nc(causal)
    in_maps = _prep_core_inputs(x, Wqkv, Wout)
    res = run_bass_kernel_spmd(nc, in_maps, core_ids=list(range(NCORES)))
    return _gather(res.results)


# ---------------------------------------------------------------------------
# Timing helper (used by test.py; not part of the graded contract)
# ---------------------------------------------------------------------------
def timed_run(x, Wqkv, Wout, mask, iters=20, reps=128, unroll=4):
    """Runs the kernel once for outputs, then times `iters` dispatches of a
    build whose body re-executes the full computation `reps` times in an
    on-device hardware loop (amortizes host/tunnel dispatch overhead, which
    is ~10ms here and unrelated to the hardware). Returns
    (y, per_rep_ns) where per_rep_ns = wall / (iters * reps)."""
    import time
    import jax
    import concourse.mybir as mybir
    from concourse import bass2jax
    from concourse.bass2jax import _bass_exec_p, install_neuronx_cc_hook, partition_id_tensor
    from jax.sharding import Mesh, PartitionSpec
    from jax.experimental.shard_map import shard_map

    x = np.asarray(x, dtype=np.float32)
    Wqkv = np.asarray(Wqkv, dtype=np.float32)
    Wout = np.asarray(Wout, dtype=np.float32)
    causal = _mask_kind(mask) == "causal"
    nc = _get_nc(causal, reps=reps, unroll=unroll)
    in_maps = _prep_core_inputs(x, Wqkv, Wout)

    install_neuronx_cc_hook()
    partition_name = nc.partition_id_tensor.name if nc.partition_id_tensor else None
    in_names, out_names, out_avals, zero_outs = [], [], [], []
    for alloc in nc.m.functions[0].allocations:
        if not isinstance(alloc, mybir.MemoryLocationSet):
            continue
        name = alloc.memorylocations[0].name
        if alloc.kind == "ExternalInput":
            if name != partition_name:
                in_names.append(name)
        elif alloc.kind == "ExternalOutput":
            out_names.append(name)
            shape = tuple(alloc.tensor_shape)
            dtype = mybir.dt.np(alloc.dtype)
            out_avals.append(jax.core.ShapedArray(shape, dtype))
            zero_outs.append(np.zeros(shape, dtype))
    n_params = len(in_names)
    all_in_names = list(in_names) + list(out_names)
    if partition_name is not None:
        all_in_names.append(partition_name)

    def _body(*args):
        operands = list(args)
        if partition_name is not None:
            operands.append(partition_id_tensor())
        outs = _bass_exec_p.bind(
            *operands,
            out_avals=tuple(out_avals),
            in_names=tuple(all_in_names),
            out_names=tuple(out_names),
            lowering_input_output_aliases=(),
            sim_require_finite=True,
            sim_require_nnan=True,
            nc=nc,
        )
        return tuple(outs)

    devices = jax.devices()[:NCORES]
    mesh = Mesh(np.asarray(devices), ("core",))
    n_outs = len(out_names)
    in_specs = (PartitionSpec("core"),) * (n_params + n_outs)
    out_specs = (PartitionSpec("core"),) * n_outs
    sharded = jax.jit(
        shard_map(_body, mesh=mesh, in_specs=in_specs, out_specs=out_specs,
                  check_rep=False),
        keep_unused=True,
    )
    per_core = [[np.asarray(m[name]) for name in in_names] for m in in_maps]
    concat_in = [
        np.concatenate([per_core[c][i] for c in range(NCORES)], axis=0)
        for i in range(n_params)
    ]
    concat_zeros = [
        np.zeros((NCORES * z.shape[0], *z.shape[1:]), z.dtype) for z in zero_outs
    ]
    from jax.sharding import NamedSharding
    shard = NamedSharding(mesh, PartitionSpec("core"))
    dev_in = [jax.device_put(a, shard) for a in concat_in]
    dev_zeros = [jax.device_put(a, shard) for a in concat_zeros]

    # warmup + correctness output
    outs = sharded(*dev_in, *dev_zeros)
    jax.block_until_ready(outs)
    results = [
        {name: np.asarray(outs[i]).reshape(NCORES, *out_avals[i].shape)[c]
         for i, name in enumerate(out_names)}
        for c in range(NCORES)
    ]
    y = _gather(results)

    t0 = time.perf_counter()
    last = None
    for _ in range(iters):
        last = sharded(*dev_in, *dev_zeros)
    jax.block_until_ready(last)
    t1 = time.perf_counter()
    per_rep_ns = (t1 - t0) / (iters * reps) * 1e9
    return y, per_rep_ns



# revision 22
# speedup vs baseline: 1.0914x; 1.0914x over previous
"""Trainium2 Bass kernel for causal multi-head attention with RoPE.

Problem: B=2, S=2048, D=1024, H=16, HD=64, fp32, causal mask.
Sharding: 8 cores = 2 (batch) x 4 (head-groups of 4 heads). Each core
computes QKV projection for its 4 heads, RoPE, causal attention, and a
partial output projection; the host sums the 4 per-batch partials and
transposes.

Kernel design (per core):
- Fused per-chunk schedule: for each 512-token chunk sc, phase A
  (QKV projection + RoPE for chunk sc) is issued back-to-back with
  phase B (causal attention for q-chunk sc over k-chunks <= sc), so
  the Tile scheduler overlaps A(sc+1)'s DVE-heavy rope with B(sc)'s
  ACT-heavy exp and keeps the PE array >90% busy mid-kernel.
- PSUM budget (8 banks): pu 2x[128,512] (QKV/V accumulators),
  mm 2x[128,1024] (scores/bcast/out-proj), oo 1x[128,1024] (attn@V
  accumulator) -- pu is separate so next-chunk QKV tiles have free
  slots while attention runs.
- Projections/out-proj in fp32r (full PE rate); Q/K pair tiles, V
  tiles and exp outputs in bf16 (same PE rate, removes the narrow-
  matmul penalty on diagonal blocks, halves SBUF).
- RoPE: 2 DVE muls (PSUM x cos/sin tables) + DVE parity stream_shuffle
  + gpsimd add (idle engine), sign-baked sin tables, W columns
  pre-permuted on host so rope pairs are adjacent partitions.
- Attention: scores matmul pairs row-packed via tile_position (0,0) /
  (64,0); exp on ACT (scale=1/8 fused, bf16 out); causal diagonal via
  N-windowed matmuls + one merged gpsimd affine_select per window;
  attn@V accumulates [65, 1024] psum per head-pair with a ones-column
  augmentation computing softmax denominators for free.
- Softmax: denominator row -> DVE reciprocal (2 alternating rec tiles)
  -> [33,512] selector matmul broadcast -> ACT copy -> DVE muls into
  fp32r staging for the output projection.
- timed_run executes `reps` copies via a hardware For_i loop
  (unroll copies per iteration; straight-line when reps==unroll) to
  amortize the ~4ms axon-tunnel dispatch overhead.
"""
import numpy as np

B, S, D, H = 2, 2048, 1024, 16
HD = D // H  # 64
NCORES = 8
HEADS_PER_CORE = 4
ROPE_BASE = 10000.0

_CACHE = {}


# ---------------------------------------------------------------------------
# TileContext workarounds for this container's walrus (1 sync-wait/inst cap)
# ---------------------------------------------------------------------------
def _make_tc_class():
    import bass_rust
    import concourse.mybir as mybir
    import concourse.tile as tile
    from concourse.vector_clock import ScopedClock, VectorClock

    def legalize_waits(nc):
        uid = 0
        for fn in nc.m.functions:
            for blk in fn.blocks:
                new_list = []
                for inst in blk.instructions:
                    si = inst.sync_info
                    waits = list(si.on_wait) if si and si.on_wait else []
                    cap = 2 if isinstance(inst, mybir.InstEventSemaphore) else 1
                    if len(waits) > cap:
                        keep, excess = waits[:cap], waits[cap:]
                        for w in excess:
                            uid += 1
                            nop = mybir.InstNoOp(
                                name=f"waitnop-{uid}-{inst.name}",
                                opcode="NoOp",
                                engine=inst.engine,
                                ins=[],
                                outs=[],
                                sync_info=bass_rust.SyncInfo(
                                    on_wait=[w], on_update=[]
                                ),
                                text_hint="split_wait",
                            )
                            new_list.append(nop)
                        si.on_wait = keep
                    new_list.append(inst)
                blk.instructions[:] = new_list

    class SplitDrainTileContext(tile.TileContext):
        def _drain_and_barrier(self, tick_clock, wait_clock):
            gc = tick_clock.global_clock
            nprocs = len(gc)
            for i in range(nprocs):
                t = gc[i]
                if t > 0:
                    nop_inst = self.nc.sync.nop(hint=f"tail_wait_p{i}", nofuse=True)
                    vec = [0] * nprocs
                    vec[i] = t
                    wait_clock.add_sem_waits(
                        nop_inst.ins, ScopedClock({None: VectorClock(vec)})
                    )
            self.nc.sync.drain()
            self.nc.all_engine_barrier()
            assert self.sems is not None
            popped = self.nc._tile_sem_poison_stack.pop()
            assert popped is self._sem_poison
            self.nc.clear_and_free_semaphores(list(self.sems.allocated().values()))
            self.nc.all_engine_barrier()

        def __exit__(self, *exc):
            ret = super().__exit__(*exc)
            if exc[0] is None:
                legalize_waits(self.nc)
            return ret

    return SplitDrainTileContext


# ---------------------------------------------------------------------------
# Bass kernel builder
# ---------------------------------------------------------------------------
def _build_nc(causal: bool, reps: int = 1, unroll: int = 1):
    import concourse.bass as bass
    import concourse.mybir as mybir

    dt = mybir.dt
    F32, F32R, BF16 = dt.float32, dt.float32r, dt.bfloat16
    AF = mybir.ActivationFunctionType
    TC = _make_tc_class()

    nc = bass.Bass(trn_type="TRN2", target_bir_lowering=False, debug=False)

    xT = nc.dram_tensor("xT", [D, S], F32R, kind="ExternalInput")
    wqk = nc.dram_tensor("wqk", [D, 512], F32R, kind="ExternalInput")
    wv = nc.dram_tensor("wv", [D, 256], F32R, kind="ExternalInput")
    wout = nc.dram_tensor("wout", [256, D], F32R, kind="ExternalInput")
    ctab = nc.dram_tensor("ctab", [128, S], F32, kind="ExternalInput")
    stab2 = nc.dram_tensor("stab2", [128, S], F32, kind="ExternalInput")
    seld = nc.dram_tensor("seld", [33, 128], F32R, kind="ExternalInput")
    onescol = nc.dram_tensor("onescol", [128, 4, 1], BF16, kind="ExternalInput")
    yT = nc.dram_tensor("yT", [D, S], F32, kind="ExternalOutput")

    NQC = S // 512  # 4 q-chunks
    NKT = S // 128  # 16 k-tiles
    SHUF_SWAP = [(i ^ 1) for i in range(32)]

    with TC(nc) as tc:
        from contextlib import ExitStack

        with ExitStack() as ctx:
            cst = ctx.enter_context(tc.tile_pool(name="cst", bufs=1))

            # --- persistent tiles
            wqk_sb = cst.tile([128, 8 * 512], F32R)
            nc.sync.dma_start(
                wqk_sb[:].rearrange("p (kt c) -> p kt c", kt=8),
                wqk.ap().rearrange("(kt p) c -> p kt c", p=128),
            )
            wv_sb = cst.tile([128, 8 * 256], F32R)
            nc.sync.dma_start(
                wv_sb[:].rearrange("p (kt c) -> p kt c", kt=8),
                wv.ap().rearrange("(kt p) c -> p kt c", p=128),
            )
            wout_sb = cst.tile([128, 2 * 1024], F32R)
            nc.sync.dma_start(
                wout_sb[:].rearrange("p (kt c) -> p kt c", kt=2),
                wout.ap().rearrange("(kt p) c -> p kt c", p=128),
            )
            ctab_sb = cst.tile([128, S], F32)
            nc.sync.dma_start(ctab_sb[:], ctab.ap())
            stab_sb = cst.tile([128, S], F32)
            nc.sync.dma_start(stab_sb[:], stab2.ap())
            sel_sb = cst.tile([33, 128], F32R)
            nc.sync.dma_start(sel_sb[:], seld.ap())
            recs = []
            for i in range(2):
                rf = cst.tile([33, 512], F32, name=f"recf{i}")
                nc.vector.memset(rf[:], 0.0)
                rr = cst.tile([33, 512], F32R, name=f"recr{i}")
                nc.vector.memset(rr[:].bitcast(F32), 0.0)
                recs.append((rf, rr))

            vaug = [
                cst.tile([128, 4 * 65], BF16, name=f"vaug{st}") for st in range(NKT)
            ]
            for st in range(NKT):
                nc.sync.dma_start(
                    vaug[st][:].rearrange("p (h c) -> p h c", h=4)[:, :, 64:65],
                    onescol.ap(),
                )

            # Q/K pair tiles (bf16): [q_p0, q_p1, k_p0, k_p1]
            qk_pair = [
                cst.tile([128, S], BF16, name=f"qk{i}") for i in range(4)
            ]

            assert reps % unroll == 0
            n_iter = reps // unroll
            loop_ctx = (
                tc.For_i(0, n_iter, 1, staggered_reset=True)
                if n_iter > 1 else None
            )
            if loop_ctx is not None:
                ctx.enter_context(loop_ctx)
            xt_pool = ctx.enter_context(tc.tile_pool(name="xt", bufs=12))
            rope_pool = ctx.enter_context(tc.tile_pool(name="rope", bufs=6))
            pT_pool = ctx.enter_context(tc.tile_pool(name="pT", bufs=8))
            stg_pool = ctx.enter_context(tc.tile_pool(name="stg", bufs=4))
            yev_pool = ctx.enter_context(tc.tile_pool(name="yev", bufs=4))
            # PSUM split (8 banks total) chosen so A(sc+1) QKV tiles have
            # slots free while B(sc) runs (A/B overlap):
            #   pu: u (QKV) + vps   [128,512]f32  = 1 bank  x2 = 2
            #   mm: scores/bcps/yps [128,1024]f32 = 2 banks x2 = 4
            #   oo: oAB accumulator [128,1024]f32 = 2 banks x1 = 2
            pu = ctx.enter_context(tc.tile_pool(name="pu", bufs=2, space="PSUM"))
            mm = ctx.enter_context(tc.tile_pool(name="mm", bufs=2, space="PSUM"))
            oo = ctx.enter_context(tc.tile_pool(name="oo", bufs=1, space="PSUM"))

            stgs = {}

            def phase_a(sc, pfx=""):
                chunks = []
                for kt in range(8):
                    xt = xt_pool.tile([128, 512], F32R, tag="xt", name=f"{pfx}xt{sc}_{kt}")
                    nc.sync.dma_start(
                        xt[:],
                        xT.ap()[kt * 128:(kt + 1) * 128, sc * 512:(sc + 1) * 512],
                    )
                    chunks.append(xt)
                ts = slice(sc * 512, (sc + 1) * 512)
                for T in range(4):
                    u = pu.tile([128, 512], F32, tag="pu", name=f"{pfx}u{sc}_{T}")
                    for kt in range(8):
                        off = kt * 512 + T * 128
                        nc.tensor.matmul(
                            u[:], wqk_sb[:, off:off + 128], chunks[kt][:],
                            start=(kt == 0), stop=(kt == 7),
                        )
                    m1 = rope_pool.tile([128, 512], F32, tag="m1", name=f"{pfx}m1_{sc}_{T}")
                    nc.vector.tensor_mul(m1[:], u[:], ctab_sb[:, ts])
                    m2p = rope_pool.tile([128, 512], F32, tag="m2p", name=f"{pfx}m2p{sc}_{T}")
                    nc.vector.tensor_mul(m2p[:], u[:], stab_sb[:, ts])
                    m2 = rope_pool.tile([128, 512], F32, tag="m2", name=f"{pfx}m2_{sc}_{T}")
                    nc.vector.stream_shuffle(m2[:], m2p[:], SHUF_SWAP)
                    nc.gpsimd.tensor_add(qk_pair[T][:, ts], m1[:], m2[:])
                for j in range(4):
                    st = 4 * sc + j
                    vps = pu.tile([128, 512], F32, tag="pu", name=f"{pfx}v{st}")
                    for kt in range(8):
                        nc.tensor.matmul(
                            vps[:, 0:256],
                            chunks[kt][:, j * 128:(j + 1) * 128],
                            wv_sb[:, kt * 256:(kt + 1) * 256],
                            start=(kt == 0), stop=(kt == 7),
                        )
                    nc.scalar.copy(
                        vaug[st][:].rearrange("p (h c) -> p h c", h=4)[:, :, 0:64],
                        vps[:, 0:256].rearrange("p (h c) -> p h c", h=4),
                    )

            def phase_b(qc, pfx=""):
                qs = slice(qc * 512, (qc + 1) * 512)
                for p in range(2):
                    q_t, k_t = qk_pair[p], qk_pair[2 + p]
                    cA = (2 * p) * 65
                    cB = (2 * p + 1) * 65
                    oAB = oo.tile([128, 1024], F32, tag="oo", name=f"{pfx}o{qc}_{p}")
                    main_kts = list(range(4 * qc)) if causal else list(range(NKT))
                    n_av = len(main_kts) + (4 if causal else 0)
                    avi = 0
                    for kt in main_kts:
                        ks = slice(kt * 128, (kt + 1) * 128)
                        sAB = mm.tile([128, 1024], F32, tag="mm", name=f"{pfx}s{qc}_{p}_{kt}")
                        nc.tensor.matmul(
                            sAB[:, 0:512], k_t[0:64, ks], q_t[0:64, qs],
                            start=True, stop=True,
                        )
                        nc.tensor.matmul(
                            sAB[:, 512:1024], k_t[64:128, ks], q_t[64:128, qs],
                            start=True, stop=True, tile_position=(64, 0),
                        )
                        pT = pT_pool.tile([128, 1024], BF16, tag="pT", name=f"{pfx}p{qc}_{p}_{kt}")
                        nc.scalar.activation(pT[:], sAB[:], AF.Exp, scale=0.125)
                        last = avi == n_av - 1
                        nc.tensor.matmul(
                            oAB[0:65, 0:512], vaug[kt][:, cA:cA + 65],
                            pT[:, 0:512], start=(avi == 0), stop=last,
                        )
                        nc.tensor.matmul(
                            oAB[0:65, 512:1024], vaug[kt][:, cB:cB + 65],
                            pT[:, 512:1024], start=(avi == 0), stop=last,
                        )
                        avi += 1
                    if causal:
                        for dl in range(4):
                            kt = 4 * qc + dl
                            w = 512 - 128 * dl
                            ks = slice(kt * 128, (kt + 1) * 128)
                            qws = slice(qc * 512 + 128 * dl, (qc + 1) * 512)
                            dAB = mm.tile([128, 1024], F32, tag="mm", name=f"{pfx}d{qc}_{p}_{dl}")
                            nc.tensor.matmul(
                                dAB[:, 0:w], k_t[0:64, ks], q_t[0:64, qws],
                                start=True, stop=True,
                            )
                            nc.tensor.matmul(
                                dAB[:, 512:512 + w], k_t[64:128, ks],
                                q_t[64:128, qws],
                                start=True, stop=True, tile_position=(64, 0),
                            )
                            pT = pT_pool.tile([128, 1024], BF16, tag="pT", name=f"{pfx}pd{qc}_{p}_{dl}")
                            src = dAB[:].rearrange("p (b c) -> p b c", b=2)[:, :, 0:w]
                            dst = pT[:].rearrange("p (b c) -> p b c", b=2)[:, :, 128 * dl:512]
                            nc.scalar.activation(dst, src, AF.Exp, scale=0.125)
                            dv = pT[:].rearrange(
                                "p (b c) -> p b c", b=2)[:, :, 128 * dl:128 * dl + 128]
                            nc.gpsimd.affine_select(
                                out=dv, in_=dv,
                                compare_op=mybir.AluOpType.is_ge,
                                fill=0.0, base=0,
                                pattern=[[0, 2], [1, 128]], channel_multiplier=-1,
                            )
                            last = avi == n_av - 1
                            nc.tensor.matmul(
                                oAB[0:65, 128 * dl:512],
                                vaug[kt][:, cA:cA + 65],
                                pT[:, 128 * dl:512],
                                start=(avi == 0), stop=last,
                            )
                            nc.tensor.matmul(
                                oAB[0:65, 512 + 128 * dl:1024],
                                vaug[kt][:, cB:cB + 65],
                                pT[:, 512 + 128 * dl:1024],
                                start=(avi == 0), stop=last,
                            )
                            avi += 1
                    # softmax denominators -> reciprocal -> broadcast
                    recf, recr = recs[p]
                    with nc.allow_low_precision(reason="softmax denom"):
                        nc.vector.reciprocal(recr[0:1, :], oAB[64:65, 0:512])
                        nc.vector.reciprocal(recr[32:33, :], oAB[64:65, 512:1024])
                    bcps = mm.tile([128, 1024], F32, tag="mm", name=f"{pfx}bc{qc}_{p}")
                    nc.tensor.matmul(
                        bcps[:, 0:512], sel_sb[:], recr[:], start=True, stop=True
                    )
                    bco = rope_pool.tile([128, 512], F32, tag="bco", name=f"{pfx}bco{qc}_{p}")
                    nc.scalar.copy(bco[:], bcps[:, 0:512])
                    stg = stg_pool.tile([128, 512], F32R, tag="stg", name=f"{pfx}stg{qc}_{p}")
                    nc.vector.tensor_mul(stg[0:64, :], oAB[0:64, 0:512], bco[0:64, :])
                    nc.vector.tensor_mul(
                        stg[64:128, :], oAB[0:64, 512:1024], bco[64:128, :]
                    )
                    stgs[(qc, p)] = stg
                # ---- output projection for this q-chunk
                for dm in range(8):
                    yps = mm.tile([128, 1024], F32, tag="mm", name=f"{pfx}y{qc}_{dm}")
                    nc.tensor.matmul(
                        yps[:, 0:512], wout_sb[:, dm * 128:dm * 128 + 128],
                        stgs[(qc, 0)][:], start=True, stop=False,
                    )
                    nc.tensor.matmul(
                        yps[:, 0:512],
                        wout_sb[:, 1024 + dm * 128:1024 + dm * 128 + 128],
                        stgs[(qc, 1)][:], start=False, stop=True,
                    )
                    yev = yev_pool.tile([128, 512], F32, tag="yev", name=f"{pfx}ye{qc}_{dm}")
                    nc.scalar.copy(yev[:], yps[:, 0:512])
                    nc.sync.dma_start(
                        yT.ap()[dm * 128:(dm + 1) * 128, qc * 512:(qc + 1) * 512],
                        yev[:],
                    )

            for cp in range(unroll):
                pfx = f"c{cp}_" if unroll > 1 else ""
                if causal:
                    # Fused: B(sc) only needs K/V chunks <= sc, so A(sc+1)
                    # can overlap B(sc) (DVE-heavy rope vs ACT-heavy exp).
                    for sc in range(NQC):
                        phase_a(sc, pfx)
                        phase_b(sc, pfx)
                else:
                    for sc in range(NQC):
                        phase_a(sc, pfx)
                    for qc in range(NQC):
                        phase_b(qc, pfx)
    return nc


# ---------------------------------------------------------------------------
# Host-side prep / gather
# ---------------------------------------------------------------------------
def _rope_tables():
    inv_freq = 1.0 / (ROPE_BASE ** (np.arange(0, HD, 2, dtype=np.float64) / HD))
    pos = np.arange(S, dtype=np.float64)
    freqs = np.outer(inv_freq, pos)  # [32, S]
    cos, sin = np.cos(freqs), np.sin(freqs)
    # pair-tile rows: r = head-local interleaved dim; m = (r % 64) // 2
    ctab = np.empty((128, S), np.float32)
    stab2 = np.empty((128, S), np.float32)
    for r in range(128):
        m = (r % 64) // 2
        ctab[r] = cos[m]
        # S[r] = -sin if r even else +sin ; stab2[r] = S[r^1]
        stab2[r] = sin[m] if (r % 2 == 0) else -sin[m]
    return ctab, stab2


def _prep_core_inputs(x, Wqkv, Wout):
    """Returns list of 8 in_map dicts."""
    perm = np.empty(HD, np.int64)
    perm[0::2] = np.arange(32)
    perm[1::2] = np.arange(32, 64)
    ctab, stab2 = _rope_tables()
    import ml_dtypes

    sel = np.zeros((33, 128), np.float32)
    sel[0, 0:64] = 1.0
    sel[32, 64:128] = 1.0
    onescol = np.ones((128, 4, 1), ml_dtypes.bfloat16)

    xT_b = [np.ascontiguousarray(x[b].T) for b in range(B)]

    in_maps = []
    for core in range(NCORES):
        b, g = divmod(core, 4)
        heads = [4 * g + j for j in range(HEADS_PER_CORE)]
        qcols = np.concatenate([h * HD + perm for h in heads])
        kcols = D + qcols
        vcols = 2 * D + np.concatenate(
            [h * HD + np.arange(HD) for h in heads]
        )
        wqk = np.ascontiguousarray(
            np.concatenate(
                [Wqkv[:, qcols], Wqkv[:, kcols]], axis=1
            )
        )  # [D, 512]
        wv = np.ascontiguousarray(Wqkv[:, vcols])  # [D, 256]
        orows = np.concatenate([h * HD + np.arange(HD) for h in heads])
        wout_c = np.ascontiguousarray(Wout[orows, :])  # [256, D]
        in_maps.append({
            "xT": xT_b[b],
            "wqk": wqk,
            "wv": wv,
            "wout": wout_c,
            "ctab": ctab,
            "stab2": stab2,
            "seld": sel,
            "onescol": onescol,
        })
    return in_maps


def _gather(results):
    y = np.empty((B, S, D), np.float32)
    for b in range(B):
        acc = results[4 * b]["yT"].astype(np.float64)
        for g in range(1, 4):
            acc += results[4 * b + g]["yT"]
        y[b] = acc.T.astype(np.float32)
    return y


def _mask_kind(mask):
    m = np.asarray(mask).reshape(S, S)
    if m.all():
        return "full"
    tri = np.tril(np.ones((S, S), dtype=bool))
    if (m == tri).all():
        return "causal"
    raise NotImplementedError("only causal (tril) or all-ones masks supported")


def _get_nc(causal, reps=1, unroll=1):
    key = ("nc", causal, reps, unroll)
    if key not in _CACHE:
        _CACHE[key] = _build_nc(causal, reps, unroll)
    return _CACHE[key]


def kernel(x, Wqkv, Wout, mask):
    from concourse.bass_utils import run_bass_kernel_spmd

    x = np.asarray(x, dtype=np.float32)
    Wqkv = np.asarray(Wqkv, dtype=np.float32)
    Wout = np.asarray(Wout, dtype=np.float32)
    causal = _mask_kind(mask) == "causal"

    nc = _get_nc(causal)
    in_maps = _prep_core_inputs(x, Wqkv, Wout)
    res = run_bass_kernel_spmd(nc, in_maps, core_ids=list(range(NCORES)))
    return _gather(res.results)


# ---------------------------------------------------------------------------
# Timing helper (used by test.py; not part of the graded contract)
# ---------------------------------------------------------------------------
def timed_run(x, Wqkv, Wout, mask, iters=20, reps=128, unroll=4):
    """Runs the kernel once for outputs, then times `iters` dispatches of a
    build whose body re-executes the full computation `reps` times in an
    on-device hardware loop (amortizes host/tunnel dispatch overhead, which
    is ~10ms here and unrelated to the hardware). Returns
    (y, per_rep_ns) where per_rep_ns = wall / (iters * reps)."""
    import time
    import jax
    import concourse.mybir as mybir
    from concourse import bass2jax
    from concourse.bass2jax import _bass_exec_p, install_neuronx_cc_hook, partition_id_tensor
    from jax.sharding import Mesh, PartitionSpec
    from jax.experimental.shard_map import shard_map

    x = np.asarray(x, dtype=np.float32)
    Wqkv = np.asarray(Wqkv, dtype=np.float32)
    Wout = np.asarray(Wout, dtype=np.float32)
    causal = _mask_kind(mask) == "causal"
    nc = _get_nc(causal, reps=reps, unroll=unroll)
    in_maps = _prep_core_inputs(x, Wqkv, Wout)

    install_neuronx_cc_hook()
    partition_name = nc.partition_id_tensor.name if nc.partition_id_tensor else None
    in_names, out_names, out_avals, zero_outs = [], [], [], []
    for alloc in nc.m.functions[0].allocations:
        if not isinstance(alloc, mybir.MemoryLocationSet):
            continue
        name = alloc.memorylocations[0].name
        if alloc.kind == "ExternalInput":
            if name != partition_name:
                in_names.append(name)
        elif alloc.kind == "ExternalOutput":
            out_names.append(name)
            shape = tuple(alloc.tensor_shape)
            dtype = mybir.dt.np(alloc.dtype)
            out_avals.append(jax.core.ShapedArray(shape, dtype))
            zero_outs.append(np.zeros(shape, dtype))
    n_params = len(in_names)
    all_in_names = list(in_names) + list(out_names)
    if partition_name is not None:
        all_in_names.append(partition_name)

    def _body(*args):
        operands = list(args)
        if partition_name is not None:
            operands.append(partition_id_tensor())
        outs = _bass_exec_p.bind(
            *operands,
            out_avals=tuple(out_avals),
            in_names=tuple(all_in_names),
            out_names=tuple(out_names),
            lowering_input_output_aliases=(),
            sim_require_finite=True,
            sim_require_nnan=True,
            nc=nc,
        )
        return tuple(outs)

    devices = jax.devices()[:NCORES]
    mesh = Mesh(np.asarray(devices), ("core",))
    n_outs = len(out_names)
    in_specs = (PartitionSpec("core"),) * (n_params + n_outs)
    out_specs = (PartitionSpec("core"),) * n_outs
    sharded = jax.jit(
        shard_map(_body, mesh=mesh, in_specs=in_specs, out_specs=out_specs,
                  check_rep=False),
        keep_unused=True,
    )
    per_core = [[np.asarray(m[name]) for name in in_names] for m in in_maps]
    concat_in = [
        np.concatenate([per_core[c][i] for c in range(NCORES)], axis=0)
        for i in range(n_params)
    ]
    concat_zeros = [
        np.zeros((NCORES * z.shape[0], *z.shape[1:]), z.dtype) for z in zero_outs
    ]
    from jax.sharding import NamedSharding
    shard = NamedSharding(mesh, PartitionSpec("core"))
    dev_in = [jax.device_put(a, shard) for a in concat_in]
    dev_zeros = [jax.device_put(a, shard) for a in concat_zeros]

    # warmup + correctness output
    outs = sharded(*dev_in, *dev_zeros)
    jax.block_until_ready(outs)
    results = [
        {name: np.asarray(outs[i]).reshape(NCORES, *out_avals[i].shape)[c]
         for i, name in enumerate(out_names)}
        for c in range(NCORES)
    ]
    y = _gather(results)

    t0 = time.perf_counter()
    last = None
    for _ in range(iters):
        last = sharded(*dev_in, *dev_zeros)
    jax.block_until_ready(last)
    t1 = time.perf_counter()
    per_rep_ns = (t1 - t0) / (iters * reps) * 1e9
    return y, per_rep_ns



# revision 24
# speedup vs baseline: 1.2767x; 1.1698x over previous
"""Trainium2 Bass kernel for causal multi-head attention with RoPE.

Problem: B=2, S=2048, D=1024, H=16, HD=64, fp32, causal mask.
Sharding: 8 cores = 2 (batch) x 4 (head-groups of 4 heads). Each core
computes QKV projection for its 4 heads, RoPE, causal attention, and a
partial output projection; the host sums the 4 per-batch partials and
transposes.

Kernel design (per core):
- Fused per-chunk schedule: for each 512-token chunk sc, phase A
  (QKV projection + RoPE for chunk sc) is issued back-to-back with
  phase B (causal attention for q-chunk sc over k-chunks <= sc), so
  the Tile scheduler overlaps A(sc+1)'s DVE-heavy rope with B(sc)'s
  ACT-heavy exp and keeps the PE array >90% busy mid-kernel.
- PSUM budget (8 banks): pu 2x[128,512] (QKV/V accumulators),
  mm 2x[128,1024] (scores/bcast/out-proj), oo 1x[128,1024] (attn@V
  accumulator) -- pu is separate so next-chunk QKV tiles have free
  slots while attention runs.
- Projections/out-proj in fp32r (full PE rate); Q/K pair tiles, V
  tiles and exp outputs in bf16 (same PE rate, removes the narrow-
  matmul penalty on diagonal blocks, halves SBUF).
- RoPE: 2 DVE muls (PSUM x cos/sin tables) + DVE parity stream_shuffle
  + gpsimd add (idle engine), sign-baked sin tables, W columns
  pre-permuted on host so rope pairs are adjacent partitions.
- Attention: scores matmul pairs row-packed via tile_position (0,0) /
  (64,0); exp on ACT (scale=1/8 fused, bf16 out); causal diagonal via
  N-windowed matmuls + one merged gpsimd affine_select per window;
  attn@V accumulates [65, 1024] psum per head-pair with a ones-column
  augmentation computing softmax denominators for free.
- Softmax: denominator row -> DVE reciprocal (2 alternating rec tiles)
  -> [33,512] selector matmul broadcast -> ACT copy -> DVE muls into
  fp32r staging for the output projection.
- timed_run executes `reps` copies via a hardware For_i loop
  (unroll copies per iteration; straight-line when reps==unroll) to
  amortize the ~4ms axon-tunnel dispatch overhead.
"""
import numpy as np

B, S, D, H = 2, 2048, 1024, 16
HD = D // H  # 64
NCORES = 8
HEADS_PER_CORE = 4
ROPE_BASE = 10000.0

_CACHE = {}


# ---------------------------------------------------------------------------
# TileContext workarounds for this container's walrus (1 sync-wait/inst cap)
# ---------------------------------------------------------------------------
def _make_tc_class():
    import bass_rust
    import concourse.mybir as mybir
    import concourse.tile as tile
    from concourse.vector_clock import ScopedClock, VectorClock

    def legalize_waits(nc):
        uid = 0
        for fn in nc.m.functions:
            for blk in fn.blocks:
                new_list = []
                for inst in blk.instructions:
                    si = inst.sync_info
                    waits = list(si.on_wait) if si and si.on_wait else []
                    cap = 2 if isinstance(inst, mybir.InstEventSemaphore) else 1
                    if len(waits) > cap:
                        keep, excess = waits[:cap], waits[cap:]
                        for w in excess:
                            uid += 1
                            nop = mybir.InstNoOp(
                                name=f"waitnop-{uid}-{inst.name}",
                                opcode="NoOp",
                                engine=inst.engine,
                                ins=[],
                                outs=[],
                                sync_info=bass_rust.SyncInfo(
                                    on_wait=[w], on_update=[]
                                ),
                                text_hint="split_wait",
                            )
                            new_list.append(nop)
                        si.on_wait = keep
                    new_list.append(inst)
                blk.instructions[:] = new_list

    class SplitDrainTileContext(tile.TileContext):
        def _drain_and_barrier(self, tick_clock, wait_clock):
            gc = tick_clock.global_clock
            nprocs = len(gc)
            for i in range(nprocs):
                t = gc[i]
                if t > 0:
                    nop_inst = self.nc.sync.nop(hint=f"tail_wait_p{i}", nofuse=True)
                    vec = [0] * nprocs
                    vec[i] = t
                    wait_clock.add_sem_waits(
                        nop_inst.ins, ScopedClock({None: VectorClock(vec)})
                    )
            self.nc.sync.drain()
            self.nc.all_engine_barrier()
            assert self.sems is not None
            popped = self.nc._tile_sem_poison_stack.pop()
            assert popped is self._sem_poison
            self.nc.clear_and_free_semaphores(list(self.sems.allocated().values()))
            self.nc.all_engine_barrier()

        def __exit__(self, *exc):
            ret = super().__exit__(*exc)
            if exc[0] is None:
                legalize_waits(self.nc)
            return ret

    return SplitDrainTileContext


# ---------------------------------------------------------------------------
# Bass kernel builder
# ---------------------------------------------------------------------------
def _build_nc(causal: bool, reps: int = 1, unroll: int = 1):
    import concourse.bass as bass
    import concourse.mybir as mybir

    dt = mybir.dt
    F32, F32R, BF16 = dt.float32, dt.float32r, dt.bfloat16
    AF = mybir.ActivationFunctionType
    TC = _make_tc_class()

    nc = bass.Bass(trn_type="TRN2", target_bir_lowering=False, debug=False)

    xT = nc.dram_tensor("xT", [D, S], F32R, kind="ExternalInput")
    wqk = nc.dram_tensor("wqk", [D, 512], F32R, kind="ExternalInput")
    wv = nc.dram_tensor("wv", [D, 256], F32R, kind="ExternalInput")
    wout = nc.dram_tensor("wout", [256, D], F32R, kind="ExternalInput")
    ctab = nc.dram_tensor("ctab", [128, S], F32, kind="ExternalInput")
    stab2 = nc.dram_tensor("stab2", [128, S], F32, kind="ExternalInput")
    seld = nc.dram_tensor("seld", [33, 128], F32R, kind="ExternalInput")
    onescol = nc.dram_tensor("onescol", [128, 4, 1], BF16, kind="ExternalInput")
    yT = nc.dram_tensor("yT", [D, S], F32, kind="ExternalOutput")

    NQC = S // 512  # 4 q-chunks
    NKT = S // 128  # 16 k-tiles
    SHUF_SWAP = [(i ^ 1) for i in range(32)]

    with TC(nc) as tc:
        from contextlib import ExitStack

        with ExitStack() as ctx:
            cst = ctx.enter_context(tc.tile_pool(name="cst", bufs=1))

            # --- persistent tiles
            wqk_sb = cst.tile([128, 8 * 512], F32R)
            nc.sync.dma_start(
                wqk_sb[:].rearrange("p (kt c) -> p kt c", kt=8),
                wqk.ap().rearrange("(kt p) c -> p kt c", p=128),
            )
            wv_sb = cst.tile([128, 8 * 256], F32R)
            nc.sync.dma_start(
                wv_sb[:].rearrange("p (kt c) -> p kt c", kt=8),
                wv.ap().rearrange("(kt p) c -> p kt c", p=128),
            )
            wout_sb = cst.tile([128, 2 * 1024], F32R)
            nc.sync.dma_start(
                wout_sb[:].rearrange("p (kt c) -> p kt c", kt=2),
                wout.ap().rearrange("(kt p) c -> p kt c", p=128),
            )
            ctab_sb = cst.tile([128, S], F32)
            nc.sync.dma_start(ctab_sb[:], ctab.ap())
            stab_sb = cst.tile([128, S], F32)
            nc.sync.dma_start(stab_sb[:], stab2.ap())
            sel_sb = cst.tile([33, 128], F32R)
            nc.sync.dma_start(sel_sb[:], seld.ap())
            recs = []
            for i in range(2):
                rr = cst.tile([33, 512], F32R, name=f"recr{i}")
                nc.vector.memset(rr[:].bitcast(F32), 0.0)
                recs.append(rr)

            vaug = [
                cst.tile([128, 4 * 65], BF16, name=f"vaug{st}") for st in range(NKT)
            ]
            for st in range(NKT):
                nc.sync.dma_start(
                    vaug[st][:].rearrange("p (h c) -> p h c", h=4)[:, :, 64:65],
                    onescol.ap(),
                )

            # Q/K pair tiles (bf16): [q_p0, q_p1, k_p0, k_p1]
            qk_pair = [
                cst.tile([128, S], BF16, name=f"qk{i}") for i in range(4)
            ]

            assert reps % unroll == 0
            n_iter = reps // unroll
            loop_ctx = (
                tc.For_i(0, n_iter, 1, staggered_reset=True)
                if n_iter > 1 else None
            )
            if loop_ctx is not None:
                ctx.enter_context(loop_ctx)
            xt_pool = ctx.enter_context(tc.tile_pool(name="xt", bufs=12))
            rope_pool = ctx.enter_context(tc.tile_pool(name="rope", bufs=6))
            pT_pool = ctx.enter_context(tc.tile_pool(name="pT", bufs=8))
            stg_pool = ctx.enter_context(tc.tile_pool(name="stg", bufs=4))
            yev_pool = ctx.enter_context(tc.tile_pool(name="yev", bufs=4))
            # PSUM split (8 banks total) chosen so A(sc+1) QKV tiles have
            # slots free while B(sc) runs (A/B overlap):
            #   pu: u (QKV) + vps   [128,512]f32  = 1 bank  x2 = 2
            #   mm: scores/bcps/yps [128,1024]f32 = 2 banks x2 = 4
            #   oo: oAB accumulator [128,1024]f32 = 2 banks x1 = 2
            pu = ctx.enter_context(tc.tile_pool(name="pu", bufs=2, space="PSUM"))
            mm = ctx.enter_context(tc.tile_pool(name="mm", bufs=2, space="PSUM"))
            oo = ctx.enter_context(tc.tile_pool(name="oo", bufs=1, space="PSUM"))

            stgs = {}

            def phase_a(sc, pfx=""):
                chunks = []
                for kt in range(8):
                    xt = xt_pool.tile([128, 512], F32R, tag="xt", name=f"{pfx}xt{sc}_{kt}")
                    nc.sync.dma_start(
                        xt[:],
                        xT.ap()[kt * 128:(kt + 1) * 128, sc * 512:(sc + 1) * 512],
                    )
                    chunks.append(xt)
                ts = slice(sc * 512, (sc + 1) * 512)
                for T in range(4):
                    u = pu.tile([128, 512], F32, tag="pu", name=f"{pfx}u{sc}_{T}")
                    for kt in range(8):
                        off = kt * 512 + T * 128
                        nc.tensor.matmul(
                            u[:], wqk_sb[:, off:off + 128], chunks[kt][:],
                            start=(kt == 0), stop=(kt == 7),
                        )
                    m1 = rope_pool.tile([128, 512], F32, tag="m1", name=f"{pfx}m1_{sc}_{T}")
                    nc.vector.tensor_mul(m1[:], u[:], ctab_sb[:, ts])
                    m2p = rope_pool.tile([128, 512], F32, tag="m2p", name=f"{pfx}m2p{sc}_{T}")
                    nc.vector.tensor_mul(m2p[:], u[:], stab_sb[:, ts])
                    m2 = rope_pool.tile([128, 512], F32, tag="m2", name=f"{pfx}m2_{sc}_{T}")
                    nc.vector.stream_shuffle(m2[:], m2p[:], SHUF_SWAP)
                    nc.gpsimd.tensor_add(qk_pair[T][:, ts], m1[:], m2[:])
                for j in range(4):
                    st = 4 * sc + j
                    vps = pu.tile([128, 512], F32, tag="pu", name=f"{pfx}v{st}")
                    for kt in range(8):
                        nc.tensor.matmul(
                            vps[:, 0:256],
                            chunks[kt][:, j * 128:(j + 1) * 128],
                            wv_sb[:, kt * 256:(kt + 1) * 256],
                            start=(kt == 0), stop=(kt == 7),
                        )
                    nc.scalar.copy(
                        vaug[st][:].rearrange("p (h c) -> p h c", h=4)[:, :, 0:64],
                        vps[:, 0:256].rearrange("p (h c) -> p h c", h=4),
                    )

            def phase_b(qc, pfx=""):
                qs = slice(qc * 512, (qc + 1) * 512)
                for p in range(2):
                    q_t, k_t = qk_pair[p], qk_pair[2 + p]
                    cA = (2 * p) * 65
                    cB = (2 * p + 1) * 65
                    oAB = oo.tile([128, 1024], F32, tag="oo", name=f"{pfx}o{qc}_{p}")
                    main_kts = list(range(4 * qc)) if causal else list(range(NKT))
                    n_av = len(main_kts) + (4 if causal else 0)
                    avi = 0
                    for kt in main_kts:
                        ks = slice(kt * 128, (kt + 1) * 128)
                        sAB = mm.tile([128, 1024], F32, tag="mm", name=f"{pfx}s{qc}_{p}_{kt}")
                        nc.tensor.matmul(
                            sAB[:, 0:512], k_t[0:64, ks], q_t[0:64, qs],
                            start=True, stop=True,
                        )
                        nc.tensor.matmul(
                            sAB[:, 512:1024], k_t[64:128, ks], q_t[64:128, qs],
                            start=True, stop=True, tile_position=(64, 0),
                        )
                        pT = pT_pool.tile([128, 1024], BF16, tag="pT", name=f"{pfx}p{qc}_{p}_{kt}")
                        nc.scalar.activation(pT[:], sAB[:], AF.Exp, scale=0.125)
                        last = avi == n_av - 1
                        nc.tensor.matmul(
                            oAB[0:65, 0:512], vaug[kt][:, cA:cA + 65],
                            pT[:, 0:512], start=(avi == 0), stop=last,
                        )
                        nc.tensor.matmul(
                            oAB[0:65, 512:1024], vaug[kt][:, cB:cB + 65],
                            pT[:, 512:1024], start=(avi == 0), stop=last,
                        )
                        avi += 1
                    if causal:
                        for dl in range(4):
                            kt = 4 * qc + dl
                            w = 512 - 128 * dl
                            ks = slice(kt * 128, (kt + 1) * 128)
                            qws = slice(qc * 512 + 128 * dl, (qc + 1) * 512)
                            dAB = mm.tile([128, 1024], F32, tag="mm", name=f"{pfx}d{qc}_{p}_{dl}")
                            nc.tensor.matmul(
                                dAB[:, 0:w], k_t[0:64, ks], q_t[0:64, qws],
                                start=True, stop=True,
                            )
                            nc.tensor.matmul(
                                dAB[:, 512:512 + w], k_t[64:128, ks],
                                q_t[64:128, qws],
                                start=True, stop=True, tile_position=(64, 0),
                            )
                            pT = pT_pool.tile([128, 1024], BF16, tag="pT", name=f"{pfx}pd{qc}_{p}_{dl}")
                            src = dAB[:].rearrange("p (b c) -> p b c", b=2)[:, :, 0:w]
                            dst = pT[:].rearrange("p (b c) -> p b c", b=2)[:, :, 128 * dl:512]
                            nc.scalar.activation(dst, src, AF.Exp, scale=0.125)
                            dv = pT[:].rearrange(
                                "p (b c) -> p b c", b=2)[:, :, 128 * dl:128 * dl + 128]
                            nc.gpsimd.affine_select(
                                out=dv, in_=dv,
                                compare_op=mybir.AluOpType.is_ge,
                                fill=0.0, base=0,
                                pattern=[[0, 2], [1, 128]], channel_multiplier=-1,
                            )
                            last = avi == n_av - 1
                            nc.tensor.matmul(
                                oAB[0:65, 128 * dl:512],
                                vaug[kt][:, cA:cA + 65],
                                pT[:, 128 * dl:512],
                                start=(avi == 0), stop=last,
                            )
                            nc.tensor.matmul(
                                oAB[0:65, 512 + 128 * dl:1024],
                                vaug[kt][:, cB:cB + 65],
                                pT[:, 512 + 128 * dl:1024],
                                start=(avi == 0), stop=last,
                            )
                            avi += 1
                    # softmax denominators -> reciprocal -> broadcast
                    recr = recs[p]
                    with nc.allow_low_precision(reason="softmax denom"):
                        nc.vector.reciprocal(recr[0:1, :], oAB[64:65, 0:512])
                        nc.vector.reciprocal(recr[32:33, :], oAB[64:65, 512:1024])
                    bcps = mm.tile([128, 1024], F32, tag="mm", name=f"{pfx}bc{qc}_{p}")
                    nc.tensor.matmul(
                        bcps[:, 0:512], sel_sb[:], recr[:], start=True, stop=True
                    )
                    bco = rope_pool.tile([128, 512], F32, tag="bco", name=f"{pfx}bco{qc}_{p}")
                    nc.scalar.copy(bco[:], bcps[:, 0:512])
                    stg = stg_pool.tile([128, 512], F32R, tag="stg", name=f"{pfx}stg{qc}_{p}")
                    nc.vector.tensor_mul(stg[0:64, :], oAB[0:64, 0:512], bco[0:64, :])
                    nc.vector.tensor_mul(
                        stg[64:128, :], oAB[0:64, 512:1024], bco[64:128, :]
                    )
                    stgs[(qc, p)] = stg
                # ---- output projection for this q-chunk
                for dm in range(8):
                    yps = mm.tile([128, 1024], F32, tag="mm", name=f"{pfx}y{qc}_{dm}")
                    nc.tensor.matmul(
                        yps[:, 0:512], wout_sb[:, dm * 128:dm * 128 + 128],
                        stgs[(qc, 0)][:], start=True, stop=False,
                    )
                    nc.tensor.matmul(
                        yps[:, 0:512],
                        wout_sb[:, 1024 + dm * 128:1024 + dm * 128 + 128],
                        stgs[(qc, 1)][:], start=False, stop=True,
                    )
                    yev = yev_pool.tile([128, 512], F32, tag="yev", name=f"{pfx}ye{qc}_{dm}")
                    nc.scalar.copy(yev[:], yps[:, 0:512])
                    nc.sync.dma_start(
                        yT.ap()[dm * 128:(dm + 1) * 128, qc * 512:(qc + 1) * 512],
                        yev[:],
                    )

            for cp in range(unroll):
                pfx = f"c{cp}_" if unroll > 1 else ""
                if causal:
                    # Fused: B(sc) only needs K/V chunks <= sc, so A(sc+1)
                    # can overlap B(sc) (DVE-heavy rope vs ACT-heavy exp).
                    for sc in range(NQC):
                        phase_a(sc, pfx)
                        phase_b(sc, pfx)
                else:
                    for sc in range(NQC):
                        phase_a(sc, pfx)
                    for qc in range(NQC):
                        phase_b(qc, pfx)
    return nc


# ---------------------------------------------------------------------------
# Host-side prep / gather
# ---------------------------------------------------------------------------
def _rope_tables():
    inv_freq = 1.0 / (ROPE_BASE ** (np.arange(0, HD, 2, dtype=np.float64) / HD))
    pos = np.arange(S, dtype=np.float64)
    freqs = np.outer(inv_freq, pos)  # [32, S]
    cos, sin = np.cos(freqs), np.sin(freqs)
    # pair-tile rows: r = head-local interleaved dim; m = (r % 64) // 2
    ctab = np.empty((128, S), np.float32)
    stab2 = np.empty((128, S), np.float32)
    for r in range(128):
        m = (r % 64) // 2
        ctab[r] = cos[m]
        # S[r] = -sin if r even else +sin ; stab2[r] = S[r^1]
        stab2[r] = sin[m] if (r % 2 == 0) else -sin[m]
    return ctab, stab2


def _prep_core_inputs(x, Wqkv, Wout):
    """Returns list of 8 in_map dicts."""
    perm = np.empty(HD, np.int64)
    perm[0::2] = np.arange(32)
    perm[1::2] = np.arange(32, 64)
    ctab, stab2 = _rope_tables()
    import ml_dtypes

    sel = np.zeros((33, 128), np.float32)
    sel[0, 0:64] = 1.0
    sel[32, 64:128] = 1.0
    onescol = np.ones((128, 4, 1), ml_dtypes.bfloat16)

    xT_b = [np.ascontiguousarray(x[b].T) for b in range(B)]

    in_maps = []
    for core in range(NCORES):
        b, g = divmod(core, 4)
        heads = [4 * g + j for j in range(HEADS_PER_CORE)]
        qcols = np.concatenate([h * HD + perm for h in heads])
        kcols = D + qcols
        vcols = 2 * D + np.concatenate(
            [h * HD + np.arange(HD) for h in heads]
        )
        wqk = np.ascontiguousarray(
            np.concatenate(
                [Wqkv[:, qcols], Wqkv[:, kcols]], axis=1
            )
        )  # [D, 512]
        wv = np.ascontiguousarray(Wqkv[:, vcols])  # [D, 256]
        orows = np.concatenate([h * HD + np.arange(HD) for h in heads])
        wout_c = np.ascontiguousarray(Wout[orows, :])  # [256, D]
        in_maps.append({
            "xT": xT_b[b],
            "wqk": wqk,
            "wv": wv,
            "wout": wout_c,
            "ctab": ctab,
            "stab2": stab2,
            "seld": sel,
            "onescol": onescol,
        })
    return in_maps


def _gather(results):
    y = np.empty((B, S, D), np.float32)
    for b in range(B):
        acc = results[4 * b]["yT"].astype(np.float64)
        for g in range(1, 4):
            acc += results[4 * b + g]["yT"]
        y[b] = acc.T.astype(np.float32)
    return y


def _mask_kind(mask):
    m = np.asarray(mask).reshape(S, S)
    if m.all():
        return "full"
    tri = np.tril(np.ones((S, S), dtype=bool))
    if (m == tri).all():
        return "causal"
    raise NotImplementedError("only causal (tril) or all-ones masks supported")


def _get_nc(causal, reps=1, unroll=1):
    key = ("nc", causal, reps, unroll)
    if key not in _CACHE:
        _CACHE[key] = _build_nc(causal, reps, unroll)
    return _CACHE[key]


def kernel(x, Wqkv, Wout, mask):
    from concourse.bass_utils import run_bass_kernel_spmd

    x = np.asarray(x, dtype=np.float32)
    Wqkv = np.asarray(Wqkv, dtype=np.float32)
    Wout = np.asarray(Wout, dtype=np.float32)
    causal = _mask_kind(mask) == "causal"

    nc = _get_nc(causal)
    in_maps = _prep_core_inputs(x, Wqkv, Wout)
    res = run_bass_kernel_spmd(nc, in_maps, core_ids=list(range(NCORES)))
    return _gather(res.results)


# ---------------------------------------------------------------------------
# Timing helper (used by test.py; not part of the graded contract)
# ---------------------------------------------------------------------------
def timed_run(x, Wqkv, Wout, mask, iters=20, reps=128, unroll=4):
    """Runs the kernel once for outputs, then times `iters` dispatches of a
    build whose body re-executes the full computation `reps` times in an
    on-device hardware loop (amortizes host/tunnel dispatch overhead, which
    is ~10ms here and unrelated to the hardware). Returns
    (y, per_rep_ns) where per_rep_ns = wall / (iters * reps)."""
    import time
    import jax
    import concourse.mybir as mybir
    from concourse import bass2jax
    from concourse.bass2jax import _bass_exec_p, install_neuronx_cc_hook, partition_id_tensor
    from jax.sharding import Mesh, PartitionSpec
    from jax.experimental.shard_map import shard_map

    x = np.asarray(x, dtype=np.float32)
    Wqkv = np.asarray(Wqkv, dtype=np.float32)
    Wout = np.asarray(Wout, dtype=np.float32)
    causal = _mask_kind(mask) == "causal"
    nc = _get_nc(causal, reps=reps, unroll=unroll)
    in_maps = _prep_core_inputs(x, Wqkv, Wout)

    install_neuronx_cc_hook()
    partition_name = nc.partition_id_tensor.name if nc.partition_id_tensor else None
    in_names, out_names, out_avals, zero_outs = [], [], [], []
    for alloc in nc.m.functions[0].allocations:
        if not isinstance(alloc, mybir.MemoryLocationSet):
            continue
        name = alloc.memorylocations[0].name
        if alloc.kind == "ExternalInput":
            if name != partition_name:
                in_names.append(name)
        elif alloc.kind == "ExternalOutput":
            out_names.append(name)
            shape = tuple(alloc.tensor_shape)
            dtype = mybir.dt.np(alloc.dtype)
            out_avals.append(jax.core.ShapedArray(shape, dtype))
            zero_outs.append(np.zeros(shape, dtype))
    n_params = len(in_names)
    all_in_names = list(in_names) + list(out_names)
    if partition_name is not None:
        all_in_names.append(partition_name)

    def _body(*args):
        operands = list(args)
        if partition_name is not None:
            operands.append(partition_id_tensor())
        outs = _bass_exec_p.bind(
            *operands,
            out_avals=tuple(out_avals),
            in_names=tuple(all_in_names),
            out_names=tuple(out_names),
            lowering_input_output_aliases=(),
            sim_require_finite=True,
            sim_require_nnan=True,
            nc=nc,
        )
        return tuple(outs)

    devices = jax.devices()[:NCORES]
    mesh = Mesh(np.asarray(devices), ("core",))
    n_outs = len(out_names)
    in_specs = (PartitionSpec("core"),) * (n_params + n_outs)
    out_specs = (PartitionSpec("core"),) * n_outs
    sharded = jax.jit(
        shard_map(_body, mesh=mesh, in_specs=in_specs, out_specs=out_specs,
                  check_rep=False),
        keep_unused=True,
    )
    per_core = [[np.asarray(m[name]) for name in in_names] for m in in_maps]
    concat_in = [
        np.concatenate([per_core[c][i] for c in range(NCORES)], axis=0)
        for i in range(n_params)
    ]
    concat_zeros = [
        np.zeros((NCORES * z.shape[0], *z.shape[1:]), z.dtype) for z in zero_outs
    ]
    from jax.sharding import NamedSharding
    shard = NamedSharding(mesh, PartitionSpec("core"))
    dev_in = [jax.device_put(a, shard) for a in concat_in]
    dev_zeros = [jax.device_put(a, shard) for a in concat_zeros]

    # warmup + correctness output
    outs = sharded(*dev_in, *dev_zeros)
    jax.block_until_ready(outs)
    results = [
        {name: np.asarray(outs[i]).reshape(NCORES, *out_avals[i].shape)[c]
         for i, name in enumerate(out_names)}
        for c in range(NCORES)
    ]
    y = _gather(results)

    t0 = time.perf_counter()
    last = None
    for _ in range(iters):
        last = sharded(*dev_in, *dev_zeros)
    jax.block_until_ready(last)
    t1 = time.perf_counter()
    per_rep_ns = (t1 - t0) / (iters * reps) * 1e9
    return y, per_rep_ns

